# revision 10
# baseline (speedup 1.0000x reference)
"""ComplexDenseSO2 Trainium2 kernel.

Computes out = (X @ conj(B)^T * w) @ B for complex X [64, 32400],
B [2048, 32400], w [2048], given as separate re/im fp32 planes.

Strategy (tensor-parallel over D across 8 cores), v2 pipelined:
  - Fold w into the first-matmul operand on the host:
    M = diag(w) @ conj(B), so mm1 output IS Y = X @ M^T.
  - Pad D 32400 -> 32768; core c owns d-slice [c*4096, (c+1)*4096).
  - K is split into NCHUNK chunks of kw columns; the whole kernel is a
    software pipeline over chunks: mm1(kc) -> AllReduce(kc) -> mm2(kc),
    with mm1 running one chunk ahead so PE/DMA never idle behind the
    collective.
  - mm1 uses TWO X stationaries, xtsA = [Xr|Xi] and xtsB = [-Xi|Xr],
    so PSUM accumulation forms the complex product directly:
    acc[j<64]  = Xr@Mr^T - Xi@Mi^T = Yr,
    acc[j>=64] = Xi@Mr^T + Xr@Mi^T = Yi.
    No per-tile transpose/combine chain at all in mm1.
  - Y chunk [j=128, kw] is AllReduced in [j, k] layout (elementwise op,
    layout agnostic), then PE-transposed once per 128-wide k block to
    build the mm2 stationaries ytA = [Yr|Yi]^T, ytB = [-Yi|Yr]^T.
  - mm2 accumulates out[128, d] over all chunks: d-chunks 0..DC_PSUM-1
    stay resident in PSUM banks across chunks; the remaining d-chunks
    go through a rotating PSUM scratch + SBUF fp32 accumulation (DVE).
  - fp16 operands use power-of-2 prescales (M*1024, B*256) to stay
    clear of fp16 subnormals; the epilogue descales by 2^-18.
"""

import sys

if "/opt/trn_rl_repo" not in sys.path:
    sys.path.insert(0, "/opt/trn_rl_repo")

import numpy as np

B_, K, D = 64, 2048, 32400
NCORES = 8
DP = 32768
DL = DP // NCORES  # 4096

COMPUTE_DT = "float16"
SCALE_M = 1024.0
SCALE_B = 256.0

NCHUNK = 4
KW = K // NCHUNK       # 512 k-columns per chunk
DC_PSUM = 5            # d-chunks of 512 kept resident in PSUM

_nc_cache = {}


def build_nc(n_cores=NCORES, k=K, dl=DL):
    import concourse.mybir as mybir
    from concourse import bacc
    import concourse.tile as tile
    from concourse.masks import make_identity

    fp = getattr(mybir.dt, COMPUTE_DT)
    f32 = mybir.dt.float32

    ndt = dl // 128        # 32 d-tiles for mm1
    nkb = KW // 128        # 4 k-blocks per chunk
    ndc = dl // 512        # 8 d-chunks for mm2
    descale = 1.0 / (SCALE_M * SCALE_B)

    nc = bacc.Bacc(
        trn_type="TRN2",
        target_bir_lowering=False,
        debug=False,
        num_devices=n_cores,
    )
    xta = nc.dram_tensor("xta", [dl, 128], fp, kind="ExternalInput")
    xtb = nc.dram_tensor("xtb", [dl, 128], fp, kind="ExternalInput")
    mtr = nc.dram_tensor("mtr", [dl, k], fp, kind="ExternalInput")
    mti = nc.dram_tensor("mti", [dl, k], fp, kind="ExternalInput")
    bnr = nc.dram_tensor("bnr", [k, dl], fp, kind="ExternalInput")
    bni = nc.dram_tensor("bni", [k, dl], fp, kind="ExternalInput")
    out = nc.dram_tensor("out", [128, dl], f32, kind="ExternalOutput")

    with tile.TileContext(nc) as tc:
        with (
            tc.tile_pool(name="sb", bufs=2) as sb,
            tc.tile_pool(name="sbx", bufs=1) as sbx,
            tc.tile_pool(name="ps", bufs=1, space="PSUM") as ps,
            tc.tile_pool(name="dram", bufs=1, space="DRAM") as dram,
        ):
            ident = sbx.tile([128, 128], fp, tag="ident")
            make_identity(nc, ident)

            # X stationaries, both packings, as 32 d-tiles side by side.
            xtsA_all = sbx.tile([128, dl], fp, tag="xtsA")
            nc.sync.dma_start(
                out=xtsA_all.rearrange("p (t j) -> p t j", j=128),
                in_=xta.ap().rearrange("(t p) j -> p t j", p=128),
            )
            xtsB_all = sbx.tile([128, dl], fp, tag="xtsB")
            nc.sync.dma_start(
                out=xtsB_all.rearrange("p (t j) -> p t j", j=128),
                in_=xtb.ap().rearrange("(t p) j -> p t j", p=128),
            )
            xtsA = [xtsA_all[:, t * 128 : (t + 1) * 128] for t in range(ndt)]
            xtsB = [xtsB_all[:, t * 128 : (t + 1) * 128] for t in range(ndt)]

            arin = [
                dram.tile([128, KW], fp, tag=f"arin{c}", name=f"arin{c}")
                for c in range(NCHUNK)
            ]
            arout = [
                dram.tile(
                    [128, KW], fp, tag=f"arout{c}", name=f"arout{c}",
                    addr_space="Shared",
                )
                for c in range(NCHUNK)
            ]

            # Persistent PSUM output banks (d-chunks 0..DC_PSUM-1).
            po = [
                ps.tile([128, 512], f32, tag=f"po{dc}", name=f"po{dc}")
                for dc in range(DC_PSUM)
            ]
            # SBUF fp32 accumulators for the remaining d-chunks.
            osb = [
                sbx.tile([128, 512], f32, tag=f"osb{dc}", name=f"osb{dc}")
                for dc in range(ndc - DC_PSUM)
            ]

            # mm2 stationaries for all k-blocks (built per chunk).
            ytA = [None] * (NCHUNK * nkb)
            ytB = [None] * (NCHUNK * nkb)

            def issue_mm2_loads(kc, sub):
                """Issue the mm2 B-row loads for chunk kc, portion sub of 2."""
                for kb in range(nkb):
                    if kb % 2 != sub:
                        continue
                    kbg = kc * nkb + kb
                    rs = slice(kbg * 128, (kbg + 1) * 128)
                    for h in range(2):
                        cs = slice(h * 2048, (h + 1) * 2048)
                        r_t = sb.tile([128, 2048], fp, tag=f"br{kb}{h}",
                                      name=f"br{kb}{h}", bufs=2)
                        nc.sync.dma_start(out=r_t, in_=bnr[rs, cs])
                        i_t = sb.tile([128, 2048], fp, tag=f"bi{kb}{h}",
                                      name=f"bi{kb}{h}", bufs=2)
                        nc.sync.dma_start(out=i_t, in_=bni[rs, cs])
                        br_tiles[(kc, kb, h)] = r_t
                        bi_tiles[(kc, kb, h)] = i_t

            br_tiles, bi_tiles = {}, {}

            for step in range(NCHUNK + 1):
                kc1 = step if step < NCHUNK else None       # mm1 chunk
                kc2 = step - 1 if step >= 1 else None       # mm2 chunk

                # ---------------- mm1(kc1) + interleaved mm2 loads ----
                if kc1 is not None:
                    ks = slice(kc1 * KW, (kc1 + 1) * KW)
                    acc = ps.tile([128, KW], f32, tag="acc", name="acc")
                    for dt in range(ndt):
                        rs = slice(dt * 128, (dt + 1) * 128)
                        mr_t = sb.tile([128, KW], fp, tag="mr", name="mr",
                                       bufs=8)
                        nc.sync.dma_start(out=mr_t, in_=mtr[rs, ks])
                        mi_t = sb.tile([128, KW], fp, tag="mi", name="mi",
                                       bufs=8)
                        nc.sync.dma_start(out=mi_t, in_=mti[rs, ks])
                        if dt in (8, 20):
                            issue_mm2_loads(kc1, 0 if dt == 8 else 1)
                        nc.tensor.matmul(acc, lhsT=xtsA[dt], rhs=mr_t,
                                         start=(dt == 0), stop=False)
                        nc.tensor.matmul(acc, lhsT=xtsB[dt], rhs=mi_t,
                                         start=False, stop=(dt == ndt - 1))
                    yc = sb.tile([128, KW], fp, tag="yc", name="yc", bufs=2)
                    nc.vector.tensor_copy(yc, acc)
                    nc.sync.dma_start(out=arin[kc1], in_=yc)
                    nc.gpsimd.collective_compute(
                        "AllReduce",
                        mybir.AluOpType.add,
                        ins=[arin[kc1].opt()],
                        outs=[arout[kc1].opt()],
                        replica_groups=[list(range(n_cores))],
                    )
                # ---------------- mm2(kc2) --------------------------
                if kc2 is None:
                    continue
                # Build stationaries from the AllReduced chunk.
                ya = sb.tile([128, KW], fp, tag="ya", name="ya", bufs=2)
                nc.sync.dma_start(out=ya, in_=arout[kc2])
                for q in range(nkb):
                    kbg = kc2 * nkb + q
                    tp = ps.tile([128, 128], fp, tag="tp", name="tp", bufs=1)
                    nc.tensor.transpose(tp, ya[:, q * 128 : (q + 1) * 128],
                                        ident)
                    a_t = sbx.tile([128, 128], fp, tag=f"ytA{kbg}",
                                   name=f"ytA{kbg}")
                    nc.scalar.copy(a_t, tp)
                    b_t = sbx.tile([128, 128], fp, tag=f"ytB{kbg}",
                                   name=f"ytB{kbg}")
                    nc.vector.tensor_scalar_mul(b_t[:, 0:64], tp[:, 64:128],
                                                -1.0)
                    nc.vector.tensor_copy(b_t[:, 64:128], tp[:, 0:64])
                    ytA[kbg] = a_t
                    ytB[kbg] = b_t

                first_c, last_c = kc2 == 0, kc2 == NCHUNK - 1
                # PSUM-resident d-chunks: kb-outer, accumulate across chunks.
                for kb in range(nkb):
                    kbg = kc2 * nkb + kb
                    st = first_c and kb == 0
                    sp = last_c and kb == nkb - 1
                    for dc in range(DC_PSUM):
                        h, q = dc // 4, dc % 4
                        qs = slice(q * 512, (q + 1) * 512)
                        nc.tensor.matmul(po[dc], lhsT=ytA[kbg],
                                         rhs=br_tiles[(kc2, kb, h)][:, qs],
                                         start=st, stop=False)
                        nc.tensor.matmul(po[dc], lhsT=ytB[kbg],
                                         rhs=bi_tiles[(kc2, kb, h)][:, qs],
                                         start=False, stop=sp)
                # SBUF-accumulated d-chunks: dc-outer, kb-inner.
                for dc in range(DC_PSUM, ndc):
                    h, q = dc // 4, dc % 4
                    qs = slice(q * 512, (q + 1) * 512)
                    pos = ps.tile([128, 512], f32, tag="pos", name="pos",
                                  bufs=1)
                    for kb in range(nkb):
                        kbg = kc2 * nkb + kb
                        nc.tensor.matmul(pos, lhsT=ytA[kbg],
                                         rhs=br_tiles[(kc2, kb, h)][:, qs],
                                         start=(kb == 0), stop=False)
                        nc.tensor.matmul(pos, lhsT=ytB[kbg],
                                         rhs=bi_tiles[(kc2, kb, h)][:, qs],
                                         start=False, stop=(kb == nkb - 1))
                    o = osb[dc - DC_PSUM]
                    if first_c:
                        nc.vector.tensor_copy(o, pos)
                    else:
                        nc.vector.tensor_add(o, o, pos)

            # ---------------- epilogue ------------------------------
            for dc in range(ndc):
                s = slice(dc * 512, (dc + 1) * 512)
                src = po[dc] if dc < DC_PSUM else osb[dc - DC_PSUM]
                o = sb.tile([128, 512], f32, tag="oep", name="oep", bufs=4)
                nc.vector.tensor_scalar_mul(o, src, descale)
                nc.sync.dma_start(out=out[:, s], in_=o)

    nc.compile()
    return nc


def _get_nc(n_cores=NCORES, k=K, dl=DL):
    key = (n_cores, k, dl)
    if key not in _nc_cache:
        _nc_cache[key] = build_nc(n_cores, k, dl)
    return _nc_cache[key]


def _prep_in_maps(X_re, X_im, bases_re, bases_im, weight_re, weight_im):
    cdt = np.float16 if COMPUTE_DT == "float16" else None
    if cdt is None:
        import ml_dtypes

        cdt = ml_dtypes.bfloat16

    f32 = np.float32
    X_re = np.asarray(X_re, f32)
    X_im = np.asarray(X_im, f32)
    bases_re = np.asarray(bases_re, f32)
    bases_im = np.asarray(bases_im, f32)
    wr = np.asarray(weight_re, f32)[:, None]
    wi = np.asarray(weight_im, f32)[:, None]

    # M = diag(w) @ conj(B): Mr = wr*Br + wi*Bi ; Mi = wi*Br - wr*Bi
    mr = (wr * bases_re + wi * bases_im) * np.float32(SCALE_M)
    mi = (wi * bases_re - wr * bases_im) * np.float32(SCALE_M)
    bsr = bases_re * np.float32(SCALE_B)
    bsi = bases_im * np.float32(SCALE_B)

    in_maps = []
    for c in range(NCORES):
        lo = c * DL
        hi = min((c + 1) * DL, D)
        n = hi - lo
        xta = np.zeros((DL, 128), cdt)
        xtb = np.zeros((DL, 128), cdt)
        if n > 0:
            xr = X_re[:, lo:hi].T.astype(cdt)
            xi = X_im[:, lo:hi].T.astype(cdt)
            xta[:n, 0:64] = xr
            xta[:n, 64:128] = xi
            xtb[:n, 0:64] = -xi
            xtb[:n, 64:128] = xr
        mtr = np.zeros((DL, K), cdt)
        mti = np.zeros((DL, K), cdt)
        bnr = np.zeros((K, DL), cdt)
        bni = np.zeros((K, DL), cdt)
        if n > 0:
            mtr[:n, :] = mr[:, lo:hi].T.astype(cdt)
            mti[:n, :] = mi[:, lo:hi].T.astype(cdt)
            bnr[:, :n] = bsr[:, lo:hi].astype(cdt)
            bni[:, :n] = bsi[:, lo:hi].astype(cdt)
        in_maps.append(
            {"xta": xta, "xtb": xtb, "mtr": mtr, "mti": mti,
             "bnr": bnr, "bni": bni}
        )
    return in_maps


def run(inputs, trace=False, trace_kwargs=None):
    """Returns (full complex64 output [64, 32400], BassKernelResults)."""
    from concourse.bass_utils import run_bass_kernel_spmd

    in_maps = _prep_in_maps(**inputs)
    nc = _get_nc()
    res = run_bass_kernel_spmd(
        nc,
        in_maps,
        core_ids=list(range(NCORES)),
        trace=trace,
        **(trace_kwargs or {}),
    )
    parts = []
    for c in range(NCORES):
        o = res.results[c]["out"]
        parts.append(o[0:64, :] + 1j * o[64:128, :].astype(np.complex64))
    full = np.concatenate(parts, axis=1)[:, :D].astype(np.complex64)
    return full, res


def kernel(**inputs) -> np.ndarray:
    out, _ = run(inputs, trace=False)
    return out


# revision 12
# speedup vs baseline: 1.1772x; 1.1772x over previous
"""ComplexDenseSO2 Trainium2 kernel.

Computes out = (X @ conj(B)^T * w) @ B for complex X [64, 32400],
B [2048, 32400], w [2048], given as separate re/im fp32 planes.

Strategy (tensor-parallel over D across 8 cores), v2 pipelined:
  - Fold w into the first-matmul operand on the host:
    M = diag(w) @ conj(B), so mm1 output IS Y = X @ M^T.
  - Pad D 32400 -> 32768; core c owns d-slice [c*4096, (c+1)*4096).
  - K is split into NCHUNK chunks of kw columns; the whole kernel is a
    software pipeline over chunks: mm1(kc) -> AllReduce(kc) -> mm2(kc),
    with mm1 running one chunk ahead so PE/DMA never idle behind the
    collective.
  - mm1 uses TWO X stationaries, xtsA = [Xr|Xi] and xtsB = [-Xi|Xr],
    so PSUM accumulation forms the complex product directly:
    acc[j<64]  = Xr@Mr^T - Xi@Mi^T = Yr,
    acc[j>=64] = Xi@Mr^T + Xr@Mi^T = Yi.
    No per-tile transpose/combine chain at all in mm1.
  - Y chunk [j=128, kw] is AllReduced in [j, k] layout (elementwise op,
    layout agnostic), then PE-transposed once per 128-wide k block to
    build the mm2 stationaries ytA = [Yr|Yi]^T, ytB = [-Yi|Yr]^T.
  - mm2 accumulates out[128, d] over all chunks: d-chunks 0..DC_PSUM-1
    stay resident in PSUM banks across chunks; the remaining d-chunks
    go through a rotating PSUM scratch + SBUF fp32 accumulation (DVE).
  - fp16 operands use power-of-2 prescales (M*1024, B*256) to stay
    clear of fp16 subnormals; the epilogue descales by 2^-18.
"""

import sys

if "/opt/trn_rl_repo" not in sys.path:
    sys.path.insert(0, "/opt/trn_rl_repo")

import numpy as np

B_, K, D = 64, 2048, 32400
NCORES = 8
DP = 32768
DL = DP // NCORES  # 4096

COMPUTE_DT = "float16"
SCALE_M = 1024.0
SCALE_B = 256.0

NCHUNK = 2
KW = K // NCHUNK       # 1024 k-columns per chunk
DC_PSUM = 4            # d-chunks of 512 kept resident in PSUM

_nc_cache = {}


def build_nc(n_cores=NCORES, k=K, dl=DL):
    import concourse.mybir as mybir
    from concourse import bacc
    import concourse.tile as tile
    from concourse.masks import make_identity

    fp = getattr(mybir.dt, COMPUTE_DT)
    f32 = mybir.dt.float32

    ndt = dl // 128        # 32 d-tiles for mm1
    nkb = KW // 128        # 8 k-blocks per chunk
    ndc = dl // 512        # 8 d-chunks for mm2
    descale = 1.0 / (SCALE_M * SCALE_B)

    nc = bacc.Bacc(
        trn_type="TRN2",
        target_bir_lowering=False,
        debug=False,
        num_devices=n_cores,
    )
    xta = nc.dram_tensor("xta", [dl, 128], fp, kind="ExternalInput")
    xtb = nc.dram_tensor("xtb", [dl, 128], fp, kind="ExternalInput")
    mtr = nc.dram_tensor("mtr", [dl, k], fp, kind="ExternalInput")
    mti = nc.dram_tensor("mti", [dl, k], fp, kind="ExternalInput")
    bnr = nc.dram_tensor("bnr", [k, dl], fp, kind="ExternalInput")
    bni = nc.dram_tensor("bni", [k, dl], fp, kind="ExternalInput")
    out = nc.dram_tensor("out", [128, dl], f32, kind="ExternalOutput")

    with tile.TileContext(nc) as tc:
        with (
            tc.tile_pool(name="sb", bufs=2) as sb,
            tc.tile_pool(name="sbx", bufs=1) as sbx,
            tc.tile_pool(name="ps", bufs=1, space="PSUM") as ps,
            tc.tile_pool(name="dram", bufs=1, space="DRAM") as dram,
        ):
            ident = sbx.tile([128, 128], fp, tag="ident")
            make_identity(nc, ident)

            # X stationaries, both packings, as 32 d-tiles side by side.
            xtsA_all = sbx.tile([128, dl], fp, tag="xtsA")
            nc.sync.dma_start(
                out=xtsA_all.rearrange("p (t j) -> p t j", j=128),
                in_=xta.ap().rearrange("(t p) j -> p t j", p=128),
            )
            xtsB_all = sbx.tile([128, dl], fp, tag="xtsB")
            nc.sync.dma_start(
                out=xtsB_all.rearrange("p (t j) -> p t j", j=128),
                in_=xtb.ap().rearrange("(t p) j -> p t j", p=128),
            )
            xtsA = [xtsA_all[:, t * 128 : (t + 1) * 128] for t in range(ndt)]
            xtsB = [xtsB_all[:, t * 128 : (t + 1) * 128] for t in range(ndt)]

            arin = [
                dram.tile([128, KW], fp, tag=f"arin{c}", name=f"arin{c}")
                for c in range(NCHUNK)
            ]
            arout = [
                dram.tile(
                    [128, KW], fp, tag=f"arout{c}", name=f"arout{c}",
                    addr_space="Shared",
                )
                for c in range(NCHUNK)
            ]

            # Persistent PSUM output banks (d-chunks 0..DC_PSUM-1).
            po = [
                ps.tile([128, 512], f32, tag=f"po{dc}", name=f"po{dc}")
                for dc in range(DC_PSUM)
            ]
            # SBUF fp32 accumulators for the remaining d-chunks.
            osb = [
                sbx.tile([128, 512], f32, tag=f"osb{dc}", name=f"osb{dc}")
                for dc in range(ndc - DC_PSUM)
            ]

            nkw = KW // 512        # 512-wide sub-blocks of a chunk
            ytA = [None] * (NCHUNK * nkb)
            ytB = [None] * (NCHUNK * nkb)
            br_tiles, bi_tiles = {}, {}

            def issue_mm1(kc):
                """mm1 for chunk kc: loads + matmuls + evac + AllReduce."""
                ks = slice(kc * KW, (kc + 1) * KW)
                acc = ps.tile([128, KW], f32, tag="acc", name="acc")
                for dt in range(ndt):
                    rs = slice(dt * 128, (dt + 1) * 128)
                    mr_t = sb.tile([128, KW], fp, tag="mr", name="mr", bufs=4)
                    nc.sync.dma_start(out=mr_t, in_=mtr[rs, ks])
                    mi_t = sb.tile([128, KW], fp, tag="mi", name="mi", bufs=4)
                    nc.sync.dma_start(out=mi_t, in_=mti[rs, ks])
                    st, sp = dt == 0, dt == ndt - 1
                    for q in range(nkw):
                        qs = slice(q * 512, (q + 1) * 512)
                        nc.tensor.matmul(acc[:, qs], lhsT=xtsA[dt],
                                         rhs=mr_t[:, qs], start=st, stop=False)
                        nc.tensor.matmul(acc[:, qs], lhsT=xtsB[dt],
                                         rhs=mi_t[:, qs], start=False, stop=sp)
                yc = sb.tile([128, KW], fp, tag="yc", name="yc", bufs=2)
                nc.vector.tensor_copy(yc, acc)
                nc.sync.dma_start(out=arin[kc], in_=yc)
                nc.gpsimd.collective_compute(
                    "AllReduce",
                    mybir.AluOpType.add,
                    ins=[arin[kc].opt()],
                    outs=[arout[kc].opt()],
                    replica_groups=[list(range(n_cores))],
                )

            def issue_mm2_loads(kc):
                """B-row loads for mm2 chunk kc: one [128, dl] tile per
                k-block per component, rolling window via pool bufs."""
                for kb in range(nkb):
                    kbg = kc * nkb + kb
                    rs = slice(kbg * 128, (kbg + 1) * 128)
                    r_t = sb.tile([128, dl], fp, tag="br", name="br", bufs=8)
                    nc.sync.dma_start(out=r_t, in_=bnr[rs, :])
                    i_t = sb.tile([128, dl], fp, tag="bi", name="bi", bufs=8)
                    nc.sync.dma_start(out=i_t, in_=bni[rs, :])
                    br_tiles[(kc, kb)] = r_t
                    bi_tiles[(kc, kb)] = i_t

            def issue_mm2(kc):
                """mm2 for chunk kc: stationaries from AR output, then
                accumulate into PSUM-resident / SBUF d-chunks."""
                ya = sb.tile([128, KW], fp, tag="ya", name="ya", bufs=2)
                nc.sync.dma_start(out=ya, in_=arout[kc])
                for q in range(nkb):
                    kbg = kc * nkb + q
                    tp = ps.tile([128, 128], fp, tag="tp", name="tp", bufs=1)
                    nc.tensor.transpose(tp, ya[:, q * 128 : (q + 1) * 128],
                                        ident)
                    a_t = sbx.tile([128, 128], fp, tag=f"ytA{kbg}",
                                   name=f"ytA{kbg}")
                    nc.scalar.copy(a_t, tp)
                    b_t = sbx.tile([128, 128], fp, tag=f"ytB{kbg}",
                                   name=f"ytB{kbg}")
                    nc.vector.tensor_scalar_mul(b_t[:, 0:64], tp[:, 64:128],
                                                -1.0)
                    nc.vector.tensor_copy(b_t[:, 64:128], tp[:, 0:64])
                    ytA[kbg] = a_t
                    ytB[kbg] = b_t

                first_c, last_c = kc == 0, kc == NCHUNK - 1
                for kb in range(nkb):
                    kbg = kc * nkb + kb
                    st = first_c and kb == 0
                    sp = last_c and kb == nkb - 1
                    for dc in range(DC_PSUM):
                        qs = slice(dc * 512, (dc + 1) * 512)
                        nc.tensor.matmul(po[dc], lhsT=ytA[kbg],
                                         rhs=br_tiles[(kc, kb)][:, qs],
                                         start=st, stop=False)
                        nc.tensor.matmul(po[dc], lhsT=ytB[kbg],
                                         rhs=bi_tiles[(kc, kb)][:, qs],
                                         start=False, stop=sp)
                for dc in range(DC_PSUM, ndc):
                    qs = slice(dc * 512, (dc + 1) * 512)
                    pos = ps.tile([128, 512], f32, tag="pos", name="pos",
                                  bufs=1)
                    for kb in range(nkb):
                        kbg = kc * nkb + kb
                        nc.tensor.matmul(pos, lhsT=ytA[kbg],
                                         rhs=br_tiles[(kc, kb)][:, qs],
                                         start=(kb == 0), stop=False)
                        nc.tensor.matmul(pos, lhsT=ytB[kbg],
                                         rhs=bi_tiles[(kc, kb)][:, qs],
                                         start=False, stop=(kb == nkb - 1))
                    o = osb[dc - DC_PSUM]
                    if first_c:
                        nc.vector.tensor_copy(o, pos)
                    else:
                        nc.vector.tensor_add(o, o, pos)

            # Software pipeline: mm1 one chunk ahead of mm2; each chunk's
            # B-row blobs queue behind that chunk's mm1 loads in DMA FIFO.
            issue_mm1(0)
            issue_mm2_loads(0)
            issue_mm1(1)
            issue_mm2_loads(1)
            issue_mm2(0)
            issue_mm2(1)

            # ---------------- epilogue ------------------------------
            for dc in range(ndc):
                s = slice(dc * 512, (dc + 1) * 512)
                src = po[dc] if dc < DC_PSUM else osb[dc - DC_PSUM]
                o = sb.tile([128, 512], f32, tag="oep", name="oep", bufs=2)
                nc.vector.tensor_scalar_mul(o, src, descale)
                nc.sync.dma_start(out=out[:, s], in_=o)

    nc.compile()
    return nc


def _get_nc(n_cores=NCORES, k=K, dl=DL):
    key = (n_cores, k, dl)
    if key not in _nc_cache:
        _nc_cache[key] = build_nc(n_cores, k, dl)
    return _nc_cache[key]


def _prep_in_maps(X_re, X_im, bases_re, bases_im, weight_re, weight_im):
    cdt = np.float16 if COMPUTE_DT == "float16" else None
    if cdt is None:
        import ml_dtypes

        cdt = ml_dtypes.bfloat16

    f32 = np.float32
    X_re = np.asarray(X_re, f32)
    X_im = np.asarray(X_im, f32)
    bases_re = np.asarray(bases_re, f32)
    bases_im = np.asarray(bases_im, f32)
    wr = np.asarray(weight_re, f32)[:, None]
    wi = np.asarray(weight_im, f32)[:, None]

    # M = diag(w) @ conj(B): Mr = wr*Br + wi*Bi ; Mi = wi*Br - wr*Bi
    mr = (wr * bases_re + wi * bases_im) * np.float32(SCALE_M)
    mi = (wi * bases_re - wr * bases_im) * np.float32(SCALE_M)
    bsr = bases_re * np.float32(SCALE_B)
    bsi = bases_im * np.float32(SCALE_B)

    in_maps = []
    for c in range(NCORES):
        lo = c * DL
        hi = min((c + 1) * DL, D)
        n = hi - lo
        xta = np.zeros((DL, 128), cdt)
        xtb = np.zeros((DL, 128), cdt)
        if n > 0:
            xr = X_re[:, lo:hi].T.astype(cdt)
            xi = X_im[:, lo:hi].T.astype(cdt)
            xta[:n, 0:64] = xr
            xta[:n, 64:128] = xi
            xtb[:n, 0:64] = -xi
            xtb[:n, 64:128] = xr
        mtr = np.zeros((DL, K), cdt)
        mti = np.zeros((DL, K), cdt)
        bnr = np.zeros((K, DL), cdt)
        bni = np.zeros((K, DL), cdt)
        if n > 0:
            mtr[:n, :] = mr[:, lo:hi].T.astype(cdt)
            mti[:n, :] = mi[:, lo:hi].T.astype(cdt)
            bnr[:, :n] = bsr[:, lo:hi].astype(cdt)
            bni[:, :n] = bsi[:, lo:hi].astype(cdt)
        in_maps.append(
            {"xta": xta, "xtb": xtb, "mtr": mtr, "mti": mti,
             "bnr": bnr, "bni": bni}
        )
    return in_maps


def run(inputs, trace=False, trace_kwargs=None):
    """Returns (full complex64 output [64, 32400], BassKernelResults)."""
    from concourse.bass_utils import run_bass_kernel_spmd

    in_maps = _prep_in_maps(**inputs)
    nc = _get_nc()
    res = run_bass_kernel_spmd(
        nc,
        in_maps,
        core_ids=list(range(NCORES)),
        trace=trace,
        **(trace_kwargs or {}),
    )
    parts = []
    for c in range(NCORES):
        o = res.results[c]["out"]
        parts.append(o[0:64, :] + 1j * o[64:128, :].astype(np.complex64))
    full = np.concatenate(parts, axis=1)[:, :D].astype(np.complex64)
    return full, res


def kernel(**inputs) -> np.ndarray:
    out, _ = run(inputs, trace=False)
    return out


# revision 13
# speedup vs baseline: 1.3714x; 1.1650x over previous
"""ComplexDenseSO2 Trainium2 kernel.

Computes out = (X @ conj(B)^T * w) @ B for complex X [64, 32400],
B [2048, 32400], w [2048], given as separate re/im fp32 planes.

Strategy (tensor-parallel over D across 8 cores), v2 pipelined:
  - Fold w into the first-matmul operand on the host:
    M = diag(w) @ conj(B), so mm1 output IS Y = X @ M^T.
  - Pad D 32400 -> 32768; core c owns d-slice [c*4096, (c+1)*4096).
  - K is split into NCHUNK chunks of kw columns; the whole kernel is a
    software pipeline over chunks: mm1(kc) -> AllReduce(kc) -> mm2(kc),
    with mm1 running one chunk ahead so PE/DMA never idle behind the
    collective.
  - mm1 uses TWO X stationaries, xtsA = [Xr|Xi] and xtsB = [-Xi|Xr],
    so PSUM accumulation forms the complex product directly:
    acc[j<64]  = Xr@Mr^T - Xi@Mi^T = Yr,
    acc[j>=64] = Xi@Mr^T + Xr@Mi^T = Yi.
    No per-tile transpose/combine chain at all in mm1.
  - Y chunk [j=128, kw] is AllReduced in [j, k] layout (elementwise op,
    layout agnostic), then PE-transposed once per 128-wide k block to
    build the mm2 stationaries ytA = [Yr|Yi]^T, ytB = [-Yi|Yr]^T.
  - mm2 accumulates out[128, d] over all chunks: d-chunks 0..DC_PSUM-1
    stay resident in PSUM banks across chunks; the remaining d-chunks
    go through a rotating PSUM scratch + SBUF fp32 accumulation (DVE).
  - fp16 operands use power-of-2 prescales (M*1024, B*256) to stay
    clear of fp16 subnormals; the epilogue descales by 2^-18.
"""

import sys

if "/opt/trn_rl_repo" not in sys.path:
    sys.path.insert(0, "/opt/trn_rl_repo")

import numpy as np

B_, K, D = 64, 2048, 32400
NCORES = 8
DP = 32768
DL = DP // NCORES  # 4096

COMPUTE_DT = "float16"
SCALE_M = 1024.0
SCALE_B = 256.0

NCHUNK = 2
KW = K // NCHUNK       # 1024 k-columns per chunk
DC_PSUM = 4            # d-chunks of 512 kept resident in PSUM

_nc_cache = {}


def build_nc(n_cores=NCORES, k=K, dl=DL):
    import concourse.mybir as mybir
    from concourse import bacc
    import concourse.tile as tile
    from concourse.masks import make_identity

    fp = getattr(mybir.dt, COMPUTE_DT)
    f32 = mybir.dt.float32

    ndt = dl // 128        # 32 d-tiles for mm1
    nkb = KW // 128        # 8 k-blocks per chunk
    ndc = dl // 512        # 8 d-chunks for mm2
    descale = 1.0 / (SCALE_M * SCALE_B)

    nc = bacc.Bacc(
        trn_type="TRN2",
        target_bir_lowering=False,
        debug=False,
        num_devices=n_cores,
    )
    xta = nc.dram_tensor("xta", [128, dl], fp, kind="ExternalInput")
    xtb = nc.dram_tensor("xtb", [128, dl], fp, kind="ExternalInput")
    mtr = nc.dram_tensor("mtr", [dl, k], fp, kind="ExternalInput")
    mti = nc.dram_tensor("mti", [dl, k], fp, kind="ExternalInput")
    bnr = nc.dram_tensor("bnr", [k, dl], fp, kind="ExternalInput")
    bni = nc.dram_tensor("bni", [k, dl], fp, kind="ExternalInput")
    out = nc.dram_tensor("out", [128, dl], f32, kind="ExternalOutput")

    with tile.TileContext(nc) as tc:
        with (
            tc.tile_pool(name="sb", bufs=2) as sb,
            tc.tile_pool(name="sbx", bufs=1) as sbx,
            tc.tile_pool(name="ps", bufs=1, space="PSUM") as ps,
            tc.tile_pool(name="dram", bufs=1, space="DRAM") as dram,
        ):
            ident = sbx.tile([128, 128], fp, tag="ident")
            make_identity(nc, ident)

            # X stationaries, both packings, as 32 d-tiles side by side.
            xtsA_all = sbx.tile([128, dl], fp, tag="xtsA")
            nc.sync.dma_start(out=xtsA_all, in_=xta.ap())
            xtsB_all = sbx.tile([128, dl], fp, tag="xtsB")
            nc.sync.dma_start(out=xtsB_all, in_=xtb.ap())
            xtsA = [xtsA_all[:, t * 128 : (t + 1) * 128] for t in range(ndt)]
            xtsB = [xtsB_all[:, t * 128 : (t + 1) * 128] for t in range(ndt)]

            arin = [
                dram.tile([128, KW], fp, tag=f"arin{c}", name=f"arin{c}")
                for c in range(NCHUNK)
            ]
            arout = [
                dram.tile(
                    [128, KW], fp, tag=f"arout{c}", name=f"arout{c}",
                    addr_space="Shared",
                )
                for c in range(NCHUNK)
            ]

            # Persistent PSUM output banks (d-chunks 0..DC_PSUM-1).
            po = [
                ps.tile([128, 512], f32, tag=f"po{dc}", name=f"po{dc}")
                for dc in range(DC_PSUM)
            ]
            # SBUF fp32 accumulators for the remaining d-chunks.
            osb = [
                sbx.tile([128, 512], f32, tag=f"osb{dc}", name=f"osb{dc}")
                for dc in range(ndc - DC_PSUM)
            ]

            nkw = KW // 512        # 512-wide sub-blocks of a chunk
            ytA = [None] * (NCHUNK * nkb)
            ytB = [None] * (NCHUNK * nkb)
            br_tiles, bi_tiles = {}, {}

            def issue_mm1(kc):
                """mm1 for chunk kc: loads + matmuls + evac + AllReduce."""
                ks = slice(kc * KW, (kc + 1) * KW)
                acc = ps.tile([128, KW], f32, tag="acc", name="acc")
                for dt in range(ndt):
                    rs = slice(dt * 128, (dt + 1) * 128)
                    mr_t = sb.tile([128, KW], fp, tag="mr", name="mr", bufs=8)
                    nc.sync.dma_start(out=mr_t, in_=mtr[rs, ks])
                    mi_t = sb.tile([128, KW], fp, tag="mi", name="mi", bufs=8)
                    nc.sync.dma_start(out=mi_t, in_=mti[rs, ks])
                    st, sp = dt == 0, dt == ndt - 1
                    for q in range(nkw):
                        qs = slice(q * 512, (q + 1) * 512)
                        nc.tensor.matmul(acc[:, qs], lhsT=xtsA[dt],
                                         rhs=mr_t[:, qs], start=st, stop=False)
                        nc.tensor.matmul(acc[:, qs], lhsT=xtsB[dt],
                                         rhs=mi_t[:, qs], start=False, stop=sp)
                yc = sb.tile([128, KW], fp, tag="yc", name="yc", bufs=2)
                nc.vector.tensor_copy(yc, acc)
                nc.sync.dma_start(out=arin[kc], in_=yc)
                nc.gpsimd.collective_compute(
                    "AllReduce",
                    mybir.AluOpType.add,
                    ins=[arin[kc].opt()],
                    outs=[arout[kc].opt()],
                    replica_groups=[list(range(n_cores))],
                )

            def issue_mm2_loads(kc):
                """B-row loads for mm2 chunk kc: one [128, dl] tile per
                k-block per component, rolling window via pool bufs."""
                for kb in range(nkb):
                    kbg = kc * nkb + kb
                    rs = slice(kbg * 128, (kbg + 1) * 128)
                    r_t = sb.tile([128, dl], fp, tag="br", name="br", bufs=4)
                    nc.sync.dma_start(out=r_t, in_=bnr[rs, :])
                    i_t = sb.tile([128, dl], fp, tag="bi", name="bi", bufs=4)
                    nc.sync.dma_start(out=i_t, in_=bni[rs, :])
                    br_tiles[(kc, kb)] = r_t
                    bi_tiles[(kc, kb)] = i_t

            def issue_yt(kc):
                """mm2 stationaries for chunk kc via XBAR-transposing DMA
                reads of the AllReduce output; ytB built on DVE."""
                for q in range(nkb):
                    kbg = kc * nkb + q
                    a_t = sbx.tile([128, 128], fp, tag=f"ytA{kbg}",
                                   name=f"ytA{kbg}")
                    nc.sync.dma_start(
                        out=a_t, in_=arout[kc][:, q * 128 : (q + 1) * 128],
                        transpose=True,
                    )
                    b_t = sbx.tile([128, 128], fp, tag=f"ytB{kbg}",
                                   name=f"ytB{kbg}")
                    nc.vector.tensor_scalar_mul(b_t[:, 0:64], a_t[:, 64:128],
                                                -1.0)
                    nc.vector.tensor_copy(b_t[:, 64:128], a_t[:, 0:64])
                    ytA[kbg] = a_t
                    ytB[kbg] = b_t

            def issue_mm2(kc):
                """mm2 for chunk kc, kb-outer: d-chunks < DC_PSUM accumulate
                in persistent PSUM across chunks; the rest go through a
                one-shot PSUM pair per (kb, dc) + DVE add into SBUF fp32."""
                first_c, last_c = kc == 0, kc == NCHUNK - 1
                for kb in range(nkb):
                    kbg = kc * nkb + kb
                    st = first_c and kb == 0
                    sp = last_c and kb == nkb - 1
                    for dc in range(DC_PSUM):
                        qs = slice(dc * 512, (dc + 1) * 512)
                        nc.tensor.matmul(po[dc], lhsT=ytA[kbg],
                                         rhs=br_tiles[(kc, kb)][:, qs],
                                         start=st, stop=False)
                        nc.tensor.matmul(po[dc], lhsT=ytB[kbg],
                                         rhs=bi_tiles[(kc, kb)][:, qs],
                                         start=False, stop=sp)
                    for dc in range(DC_PSUM, ndc):
                        qs = slice(dc * 512, (dc + 1) * 512)
                        rot = ps.tile([128, 512], f32, tag="rot", name="rot",
                                      bufs=2)
                        nc.tensor.matmul(rot, lhsT=ytA[kbg],
                                         rhs=br_tiles[(kc, kb)][:, qs],
                                         start=True, stop=False)
                        nc.tensor.matmul(rot, lhsT=ytB[kbg],
                                         rhs=bi_tiles[(kc, kb)][:, qs],
                                         start=False, stop=True)
                        o = osb[dc - DC_PSUM]
                        if first_c and kb == 0:
                            nc.vector.tensor_copy(o, rot)
                        else:
                            nc.vector.tensor_add(o, o, rot)

            # Software pipeline: both mm1 chunks (and both AllReduces)
            # issue first in DMA FIFO; B-row blobs stream after, consumed
            # just-in-time; stationary transposes slot in between.
            issue_mm1(0)
            issue_mm1(1)
            issue_yt(0)
            issue_mm2_loads(0)
            issue_yt(1)
            issue_mm2_loads(1)
            issue_mm2(0)
            issue_mm2(1)

            # ---------------- epilogue ------------------------------
            for dc in range(ndc):
                s = slice(dc * 512, (dc + 1) * 512)
                src = po[dc] if dc < DC_PSUM else osb[dc - DC_PSUM]
                o = sb.tile([128, 512], f32, tag="oep", name="oep", bufs=2)
                nc.vector.tensor_scalar_mul(o, src, descale)
                nc.sync.dma_start(out=out[:, s], in_=o)

    nc.compile()
    return nc


def _get_nc(n_cores=NCORES, k=K, dl=DL):
    key = (n_cores, k, dl)
    if key not in _nc_cache:
        _nc_cache[key] = build_nc(n_cores, k, dl)
    return _nc_cache[key]


def _prep_in_maps(X_re, X_im, bases_re, bases_im, weight_re, weight_im):
    cdt = np.float16 if COMPUTE_DT == "float16" else None
    if cdt is None:
        import ml_dtypes

        cdt = ml_dtypes.bfloat16

    f32 = np.float32
    X_re = np.asarray(X_re, f32)
    X_im = np.asarray(X_im, f32)
    bases_re = np.asarray(bases_re, f32)
    bases_im = np.asarray(bases_im, f32)
    wr = np.asarray(weight_re, f32)[:, None]
    wi = np.asarray(weight_im, f32)[:, None]

    # M = diag(w) @ conj(B): Mr = wr*Br + wi*Bi ; Mi = wi*Br - wr*Bi
    mr = (wr * bases_re + wi * bases_im) * np.float32(SCALE_M)
    mi = (wi * bases_re - wr * bases_im) * np.float32(SCALE_M)
    bsr = bases_re * np.float32(SCALE_B)
    bsi = bases_im * np.float32(SCALE_B)

    in_maps = []
    for c in range(NCORES):
        lo = c * DL
        hi = min((c + 1) * DL, D)
        n = hi - lo
        # SBUF images [128, DL]: img[p, t*128 + j] = src[t*128 + p, j]
        xta = np.zeros((DL, 128), np.float32)
        xtb = np.zeros((DL, 128), np.float32)
        if n > 0:
            xr = X_re[:, lo:hi].T.astype(np.float32)
            xi = X_im[:, lo:hi].T.astype(np.float32)
            xta[:n, 0:64] = xr
            xta[:n, 64:128] = xi
            xtb[:n, 0:64] = -xi
            xtb[:n, 64:128] = xr
        xta = np.ascontiguousarray(
            xta.reshape(DL // 128, 128, 128).transpose(1, 0, 2)
            .reshape(128, DL)).astype(cdt)
        xtb = np.ascontiguousarray(
            xtb.reshape(DL // 128, 128, 128).transpose(1, 0, 2)
            .reshape(128, DL)).astype(cdt)
        mtr = np.zeros((DL, K), cdt)
        mti = np.zeros((DL, K), cdt)
        bnr = np.zeros((K, DL), cdt)
        bni = np.zeros((K, DL), cdt)
        if n > 0:
            mtr[:n, :] = mr[:, lo:hi].T.astype(cdt)
            mti[:n, :] = mi[:, lo:hi].T.astype(cdt)
            bnr[:, :n] = bsr[:, lo:hi].astype(cdt)
            bni[:, :n] = bsi[:, lo:hi].astype(cdt)
        in_maps.append(
            {"xta": xta, "xtb": xtb, "mtr": mtr, "mti": mti,
             "bnr": bnr, "bni": bni}
        )
    return in_maps


def run(inputs, trace=False, trace_kwargs=None):
    """Returns (full complex64 output [64, 32400], BassKernelResults)."""
    from concourse.bass_utils import run_bass_kernel_spmd

    in_maps = _prep_in_maps(**inputs)
    nc = _get_nc()
    res = run_bass_kernel_spmd(
        nc,
        in_maps,
        core_ids=list(range(NCORES)),
        trace=trace,
        **(trace_kwargs or {}),
    )
    parts = []
    for c in range(NCORES):
        o = res.results[c]["out"]
        parts.append(o[0:64, :] + 1j * o[64:128, :].astype(np.complex64))
    full = np.concatenate(parts, axis=1)[:, :D].astype(np.complex64)
    return full, res


def kernel(**inputs) -> np.ndarray:
    out, _ = run(inputs, trace=False)
    return out
